# revision 50
# baseline (speedup 1.0000x reference)
"""Multi-head attention (B=1, S=4096, D=768, H=12, Hd=64) on 8 trn2 cores.

Sharding (v3): 4 head-groups (3 heads = 192 dims, Megatron column-split
wq/wk/wv, row-split ww) x 2 query-chunks (2048 rows).  core = g*2 + c.
Each core returns a partial output [2048, 768]; host sums the 4 group
partials per chunk and adds (bv @ ww.T + bw).  Fewer heads per group halves
the replicated K/V projection work versus a 2x6 split, which keeps the PE
under the ACT (exp) budget in every phase.

Pipeline design (ACT-bound fused schedule):
  - Projections / scores in bf16.  K/Q weights are zero-padded to 256 cols
    so the 3 heads pack into 2 K=128 pairs (head 2 rides rows 0-63 of pair
    1, upper half zero).
  - attnV in fp8e4 DoubleRow (2 key-tiles per instruction): V8 holds
    fp8(32*V) rows per key with a ones column at 64 and 63 pad cols (dual
    fp8 ldweights needs all 128 weight columns); pt = fp8(8*exp(s/8))
    written directly by the ACT exp.  y6 = 32*out; ww is pre-divided by 32
    on the host.
  - The exp stream on ACT is the binding engine.  The key axis runs in
    NSPLIT=4 quarters; each quarter has 12 rounds (qh 0-3, head 0-2) of 4
    cs=2 chunks through a 3-deep [128,2,512] psum rotation.  attnV pairs
    and the per-round spill are queued and emitted one chunk late so the
    in-order PE queue never blocks on a just-issued exp.  K/V projection
    blocks, the Q projection, and the out-projection run as paced filler
    pieces inside the quarters; `require` force-emits any producer a
    consumer needs (program order = dependency order).  attnV accumulates
    per-quarter in a shared 2-buf psum pool and spills into acc (f32).
  - psum: scores 3x2 banks + 2 shared o/filler banks = 8.
"""

import sys

if "/opt/trn_rl_repo" not in sys.path:
    sys.path.insert(0, "/opt/trn_rl_repo")

import math
import os
from collections import deque

import numpy as np
import ml_dtypes

import concourse.bacc as bacc
import concourse.mybir as mybir
import concourse.tile as tile
from concourse.bass_utils import run_bass_kernel_spmd
from concourse.vector_clock import ScopedClock

F32 = mybir.dt.float32
BF = mybir.dt.bfloat16
F8 = mybir.dt.float8e4
AF = mybir.ActivationFunctionType
DR = mybir.MatmulPerfMode.DoubleRow

S = 4096          # sequence length
D = 768           # model dim
NG = 4            # head groups (cores axis 1)
NC = 2            # query chunks (cores axis 2)
DH = D // NG      # dims per group = 192
DHP = 256         # padded dims (2 K=128 pairs)
NPR = 2           # K=128 pairs per group
NH = 3            # heads per group
SQ = S // NC      # queries per core = 2048
NQH = SQ // 512   # 512-query rounds per head = 4
KO = D // 128     # contraction subtiles = 6
NJ = S // 128     # key tiles = 32
SCALE = 0.125     # 1/sqrt(64)
LN8 = float(math.log(8.0))
VSCALE = 32.0     # folded into wv (1/VSCALE into ww); keeps |VSCALE*v| well
                  # below the TRN e4m3 max of 240 (DVE f32->fp8 conversion
                  # overflows instead of saturating)

PROBE = os.environ.get("PROBE", "")   # unused; kept for probe scripts

NSPLIT = 4        # key-axis quarters
JQ = NJ // NSPLIT           # j-tiles per quarter = 8
NPAIR = JQ // 2             # DoubleRow pairs per round-quarter = 4
QCHUNKS = [2, 2, 2, 2]      # exp chunk sizes covering JQ j-tiles
SC_BUFS = 3
SCW = max(QCHUNKS)
ROUNDS = [(qh, h) for qh in range(NQH) for h in range(NH)]  # 12 per quarter

_PATCHED = False


def _patch_drain():
    """walrus in this container rejects >1 sync-wait per instruction
    ("Too many sync wait commands").  TileContext's tail drain aggregates one
    wait per live tile semaphore; redistribute them one-per-nop.  (Bacc's
    generate_event_semaphores handles the rest of the kernel.)"""
    global _PATCHED
    if _PATCHED:
        return
    _PATCHED = True

    def _drain_and_barrier(self, tick_clock, wait_clock):
        nc = self.nc
        drain_inst = nc.sync.drain()
        wait_clock.add_sem_waits(
            drain_inst.ins, ScopedClock({None: tick_clock.global_clock})
        )
        si = drain_inst.ins.sync_info
        waits = list(si.on_wait) if si is not None else []
        if len(waits) > 1:
            drain_inst.ins.sync_info = mybir.SyncInfo(
                on_wait=[waits[0]], on_update=list(si.on_update)
            )
            for w in waits[1:]:
                nop = nc.sync.nop(nofuse=True)
                nop.ins.sync_info = mybir.SyncInfo(on_wait=[w], on_update=[])
        nc.all_engine_barrier()
        assert self.sems is not None
        popped = nc._tile_sem_poison_stack.pop()
        assert popped is self._sem_poison
        nc.clear_and_free_semaphores(list(self.sems.allocated().values()))
        nc.all_engine_barrier()

    tile.TileContext._drain_and_barrier = _drain_and_barrier


def build_nc(loop_n=None, debug=False):
    _patch_drain()
    nc = bacc.Bacc("TRN2", target_bir_lowering=False)

    xT = nc.dram_tensor("xT", [D, S], BF, kind="ExternalInput")
    xqT = nc.dram_tensor("xqT", [D, SQ], BF, kind="ExternalInput")
    wqT = nc.dram_tensor("wqT", [D, DHP], BF, kind="ExternalInput")  # padded
    wkT = nc.dram_tensor("wkT", [D, DHP], BF, kind="ExternalInput")  # padded
    wvT = nc.dram_tensor("wvT", [D, DH], BF, kind="ExternalInput")   # x VSCALE
    wwT = nc.dram_tensor("wwT", [DH, D], BF, kind="ExternalInput")   # / VSCALE
    bq = nc.dram_tensor("bq", [128, NPR], F32, kind="ExternalInput")
    bk = nc.dram_tensor("bk", [128, NPR], F32, kind="ExternalInput")
    out = nc.dram_tensor("out", [SQ, D], F32, kind="ExternalOutput")

    xT_r = xT.rearrange("(ko p) n -> p ko n", p=128)
    xqT_r = xqT.rearrange("(ko p) n -> p ko n", p=128)
    wqT_r = wqT.rearrange("(ko p) m -> p ko m", p=128)
    wkT_r = wkT.rearrange("(ko p) m -> p ko m", p=128)
    wvT_r = wvT.rearrange("(ko p) m -> p ko m", p=128)
    ww6_r = wwT.rearrange("(h l) o -> l h o", l=64)   # [64, 3, 768]

    with tile.TileContext(nc) as tc:
        import contextlib

        with contextlib.ExitStack() as ctx:
            if loop_n is not None:
                ctx.enter_context(tc.For_i(0, loop_n, 1))
            persist = ctx.enter_context(tc.tile_pool(name="persist", bufs=1))
            KT = persist.tile([128, NPR, S], BF)        # 16KB/part
            QTz = persist.tile([128, NH, SQ], BF)       # 12KB/part
            V8 = persist.tile([128, NH, NJ, 128], F8)   # 12KB/part
            acc = persist.tile([128, 12, 512], F32)     # 24KB/part
            ptr = persist.tile([128, 2, 6, 512], F8)    # exp rings, 6KB/part
            y6 = persist.tile([128, NH, SQ], BF)        # 12KB/part
            ww6 = persist.tile([128, NH, D], BF)        # 4.5KB/part
            lnb = persist.tile([128, 1], F32)
            bq_sb = persist.tile([128, NPR], F32)
            bk_sb = persist.tile([128, NPR], F32)

            w_pool = ctx.enter_context(tc.tile_pool(name="w", bufs=1))
            wk_sb = w_pool.tile([128, KO, DHP], BF)
            wv_sb = w_pool.tile([128, KO, DH], BF)
            wq_sb = w_pool.tile([128, KO, DHP], BF)

            xs = ctx.enter_context(tc.tile_pool(name="xs", bufs=3))
            ob_pool = ctx.enter_context(tc.tile_pool(name="ob", bufs=2))
            bc_pool = ctx.enter_context(tc.tile_pool(name="bc", bufs=2))

            sc_pool = ctx.enter_context(
                tc.tile_pool(name="sc", bufs=SC_BUFS, space="PSUM"))
            # shared 2-buf pool for attnV accumulators AND filler psum
            ok_pool = ctx.enter_context(
                tc.tile_pool(name="ok", bufs=2, space="PSUM"))

            # ---------------- init + weight DMA ----------------
            nc.sync.dma_start(wk_sb[:], wkT_r[:])
            nc.sync.dma_start(bk_sb[:], bk[:])
            nc.vector.memset(lnb[:], LN8)
            # zero inits on gpsimd (idle early); QTz zero halves kill the
            # cross-head term of the pair-packed K=128 scores matmuls
            nc.gpsimd.memset(QTz[64:128, 0, :], 0.0)
            nc.gpsimd.memset(QTz[0:64, 1, :], 0.0)
            nc.gpsimd.memset(QTz[64:128, 2, :], 0.0)
            # upper halves zero: out-proj runs K=128 on single heads
            nc.gpsimd.memset(y6[64:128, :, :], 0.0)
            nc.gpsimd.memset(ww6[64:128, :, :], 0.0)

            # ---------------- pieces ----------------
            ready = set()
            ps_rotate = {"on": False, "i": 0}

            def next_ps(name):
                if not ps_rotate["on"]:
                    return ok_pool.tile([128, 512], F32, tag="ok", name=name)
                i = ps_rotate["i"] = ps_rotate["i"] + 1
                if i % 2 == 0:
                    return ok_pool.tile([128, 512], F32, tag="ok", name=name)
                sc = sc_pool.tile([128, SCW, 512], F32, tag="sc", name=name)
                return sc[:, 0, :]

            xb_tiles = {}

            def piece_dma_block(n):
                def go():
                    xb = xs.tile([128, KO, 512], BF, tag="xb", name=f"xb{n}")
                    xb_tiles[n] = xb
                    nc.sync.dma_start(xb[:], xT_r[:, :, n * 512:(n + 1) * 512])
                    # fp8 pad cols + ones col for this block's j-tiles
                    nc.gpsimd.memset(V8[:, :, 4 * n:4 * n + 4, 64:128], 0.0)
                    nc.gpsimd.memset(V8[:, :, 4 * n:4 * n + 4, 64:65], 1.0)
                return go

            def piece_k(n, p):
                def go():
                    xb = xb_tiles[n]
                    ps = next_ps(f"psk{n}_{p}")
                    for ko in range(KO):
                        nc.tensor.matmul(
                            ps[:], wk_sb[:, ko, p * 128:(p + 1) * 128],
                            xb[:, ko, :],
                            start=(ko == 0), stop=(ko == KO - 1),
                        )
                    nc.vector.tensor_scalar_add(
                        KT[:, p, n * 512:(n + 1) * 512], ps[:],
                        bk_sb[:, p:p + 1],
                    )
                    ready.add(("K", n, p))
                return go

            def piece_v(n, j4):
                def go():
                    xb = xb_tiles[n]
                    ps = next_ps(f"psv{n}_{j4}")
                    for ko in range(KO):
                        nc.tensor.matmul(
                            ps[:, :DH],
                            xb[:, ko, j4 * 128:(j4 + 1) * 128],
                            wv_sb[:, ko, :],
                            start=(ko == 0), stop=(ko == KO - 1),
                        )
                    nc.vector.tensor_copy(
                        V8[:, :, 4 * n + j4, 0:64],
                        ps[:, 0:DH].rearrange("l (h c) -> l h c", c=64),
                    )
                    ready.add(("V", 4 * n + j4))
                return go

            def block_pieces(n):
                ps = [piece_dma_block(n)]
                for p in range(NPR):
                    ps.append(piece_k(n, p))
                for j4 in range(4):
                    ps.append(piece_v(n, j4))
                return ps

            xq_tiles = {}

            def piece_qproj_dma(nq):
                def go():
                    xqb = xs.tile([128, KO, 512], BF, tag="xb",
                                  name=f"xqb{nq}")
                    xq_tiles[nq] = xqb
                    nc.sync.dma_start(
                        xqb[:], xqT_r[:, :, nq * 512:(nq + 1) * 512])
                return go

            def piece_qproj_p(nq, p):
                def go():
                    xqb = xq_tiles[nq]
                    nqs = slice(nq * 512, (nq + 1) * 512)
                    psq = next_ps(f"psq{nq}_{p}")
                    for ko in range(KO):
                        nc.tensor.matmul(
                            psq[:], wq_sb[:, ko, p * 128:(p + 1) * 128],
                            xqb[:, ko, :],
                            start=(ko == 0), stop=(ko == KO - 1),
                        )
                    nc.vector.tensor_scalar_add(
                        QTz[0:64, 2 * p, nqs], psq[0:64, :],
                        bq_sb[0:64, p:p + 1],
                    )
                    if p == 0:
                        nc.vector.tensor_scalar_add(
                            QTz[64:128, 1, nqs], psq[64:128, :],
                            bq_sb[64:128, p:p + 1],
                        )
                    ready.add(("Q", nq, p))
                return go

            ob_tiles = {}

            def piece_op(m, n0, nw):
                # out-projection for m-tile cols [n0, n0+nw), all 3 heads
                def go():
                    ms = slice(m * 128, (m + 1) * 128)
                    ps = next_ps(f"op{m}_{n0}")
                    for h in range(NH):
                        nc.tensor.matmul(
                            ps[:, :nw],
                            y6[:, h, ms],
                            ww6[:, h, n0:n0 + nw],
                            start=(h == 0), stop=(h == NH - 1),
                        )
                    if n0 == 0:
                        ob_tiles[m] = ob_pool.tile(
                            [128, D], F32, tag="ob", name=f"ob{m}")
                    ob = ob_tiles[m]
                    nc.vector.tensor_copy(ob[:, n0:n0 + nw], ps[:, :nw])
                    if n0 + nw == D:
                        nc.sync.dma_start(out[ms, :], ob[:])
                return go

            COST_DMA = 200
            COST_K = KO * 512
            COST_V = KO * DH
            COST_QP = KO * 512

            # ---------------- lead-in ----------------
            ps_rotate["on"] = True
            pieces0 = block_pieces(0)
            pieces0[0]()                        # xb0 DMA first in queue
            nc.sync.dma_start(wv_sb[:], wvT_r[:])
            nc.sync.dma_start(wq_sb[:], wqT_r[:])
            nc.sync.dma_start(bq_sb[:], bq[:])
            for piece in pieces0[1:]:
                piece()
            piece_qproj_dma(0)()
            piece_qproj_p(0, 0)()
            piece_qproj_p(0, 1)()
            for piece in block_pieces(1):
                piece()

            # ---------------- fused attention quarters ----------------
            ps_rotate["on"] = False
            NQ = int(os.environ.get("NQ", NSPLIT))  # timing probe: truncate
            pend = deque()     # (chunk-emitted, closure) attnV/spill queue
            gchunk = [0]
            fillers = deque()  # (closure, cost, min_round_gate) — persistent

            def require(marker):
                # force-emit fillers until the producer of `marker` has been
                # emitted (program order = dependency order)
                while marker not in ready and fillers:
                    fillers.popleft()[0]()
                assert marker in ready, f"missing producer {marker}"

            for q in range(NQ):
                if q == 0:
                    # Q for qh1 first (needed at round 3), then block 2
                    # (quarter 1's first j-tiles), then the later Q groups
                    # interleaved with block 3
                    fillers.append((piece_qproj_dma(1), COST_DMA, 0))
                    fillers.append((piece_qproj_p(1, 0), COST_QP, 0))
                    fillers.append((piece_qproj_p(1, 1), COST_QP, 0))
                    fillers.append((piece_dma_block(2), COST_DMA, 0))
                    for p in range(NPR):
                        fillers.append((piece_k(2, p), COST_K, 0))
                    fillers.append((piece_qproj_dma(2), COST_DMA, 0))
                    fillers.append((piece_qproj_p(2, 0), COST_QP, 0))
                    fillers.append((piece_qproj_p(2, 1), COST_QP, 0))
                    for j4 in range(4):
                        fillers.append((piece_v(2, j4), COST_V, 0))
                    fillers.append((piece_dma_block(3), COST_DMA, 0))
                    for p in range(NPR):
                        fillers.append((piece_k(3, p), COST_K, 0))
                    fillers.append((piece_qproj_dma(3), COST_DMA, 0))
                    fillers.append((piece_qproj_p(3, 0), COST_QP, 0))
                    fillers.append((piece_qproj_p(3, 1), COST_QP, 0))
                    for j4 in range(4):
                        fillers.append((piece_v(3, j4), COST_V, 0))
                elif q < NSPLIT - 1:
                    if q == 1:
                        nc.sync.dma_start(ww6[0:64, :, :], ww6_r[:])
                    na, nb = 2 * q + 2, 2 * q + 3
                    # both DMAs and K pieces ahead of the V pieces: the next
                    # quarter's scores depend on K, and the second DMA
                    # overlaps the first block's compute
                    fillers.append((piece_dma_block(na), COST_DMA, 0))
                    for p in range(NPR):
                        fillers.append((piece_k(na, p), COST_K, 0))
                    fillers.append((piece_dma_block(nb), COST_DMA, 0))
                    for j4 in range(4):
                        fillers.append((piece_v(na, j4), COST_V, 0))
                    for p in range(NPR):
                        fillers.append((piece_k(nb, p), COST_K, 0))
                    for j4 in range(4):
                        fillers.append((piece_v(nb, j4), COST_V, 0))
                else:
                    # out-proj for qh 0-2 as fillers gated on their rounds;
                    # qh 3 runs in the tail
                    for m in range(12):
                        for (n0, nw) in ((0, 512), (512, 256)):
                            fillers.append(
                                (piece_op(m, n0, nw), NH * nw,
                                 3 * (m // 4) + 3))
                total_cost = sum(c for _, c, _ in fillers)
                n_slots = 12 * len(QCHUNKS)
                budget_rate = total_cost / n_slots
                budget = 0.0

                for r, (qh, h) in enumerate(ROUNDS):
                    kp = h >> 1
                    qs = slice(qh * 512, (qh + 1) * 512)
                    ring = ptr[:, r % 2, :, :]
                    require(("Q", qh, kp))
                    ohold = {}

                    def mk_pair(t_l, q=q, r=r, h=h, ring=ring, ohold=ohold):
                        def go():
                            jg = q * JQ + 2 * t_l
                            require(("V", jg))
                            require(("V", jg + 1))
                            if t_l == 0:
                                ohold["t"] = ok_pool.tile(
                                    [128, 512], F32, tag="ok",
                                    name=f"o{q}_{r}")
                            nc.tensor.matmul(
                                ohold["t"][:],
                                V8[:, h, jg:jg + 2, :],
                                ring[:, (2 * t_l) % 6:(2 * t_l) % 6 + 2, :],
                                start=(t_l == 0), stop=(t_l == NPAIR - 1),
                                perf_mode=DR,
                            )
                        return go

                    def mk_spill(q=q, r=r, h=h, qs=qs, ohold=ohold):
                        def go():
                            o_ps = ohold["t"]
                            if q == 0:
                                nc.vector.tensor_copy(
                                    acc[0:65, r, :], o_ps[0:65, :])
                            else:
                                nc.vector.tensor_add(
                                    acc[0:65, r, :], o_ps[0:65, :],
                                    acc[0:65, r, :])
                            if q == NSPLIT - 1 and h == NH - 1:
                                # normalize all 3 heads of this query-half
                                # in one batched chain (acc slots r-2..r)
                                dn = bc_pool.tile([1, 3, 512], F32, tag="dn",
                                                  name=f"dn{r}")
                                nc.vector.tensor_copy(
                                    dn[:], acc[64:65, r - 2:r + 1, :])
                                bc = bc_pool.tile([64, 3, 512], F32, tag="bc",
                                                  name=f"bc{r}")
                                nc.gpsimd.partition_broadcast(
                                    bc[:], dn[:], channels=64)
                                nc.vector.reciprocal(bc[:], bc[:])
                                nc.vector.tensor_mul(
                                    y6[0:64, :, qs],
                                    acc[0:64, r - 2:r + 1, :], bc[:])
                        return go

                    jc = 0
                    pair_emitted = 0
                    for c, cs in enumerate(QCHUNKS):
                        for t in range(cs):
                            require(("K", (q * JQ + jc + t) // 4, kp))
                        sc = sc_pool.tile([128, SCW, 512], F32, tag="sc")
                        for t in range(cs):
                            j = q * JQ + jc + t
                            nc.tensor.matmul(
                                sc[:, t, :],
                                KT[:, kp, j * 128:(j + 1) * 128],
                                QTz[:, h, qs],
                                start=True, stop=True,
                            )
                        slot = jc % 6
                        nc.scalar.activation(
                            ring[:, slot:slot + cs, :], sc[:, :cs, :],
                            AF.Exp, scale=SCALE, bias=lnb[:],
                        )
                        jc += cs
                        # queue attnV pairs completed by this chunk's exp;
                        # they pop a chunk later (possibly in the next round
                        # or quarter) so the in-order PE queue never blocks
                        # on an exp that was just issued
                        while 2 * (pair_emitted + 1) <= jc:
                            pend.append((gchunk[0], mk_pair(pair_emitted)))
                            pair_emitted += 1
                        if pair_emitted == NPAIR:
                            pend.append((gchunk[0], mk_spill()))
                            pair_emitted += 1
                        gchunk[0] += 1
                        while pend and pend[0][0] < gchunk[0] - 1:
                            pend.popleft()[1]()
                        if not fillers or fillers[0][2] <= r:
                            budget += budget_rate
                        while fillers and budget >= fillers[0][1] \
                                and fillers[0][2] <= r:
                            piece, cost, _ = fillers.popleft()
                            piece()
                            budget -= cost

            while pend:
                pend.popleft()[1]()
            while fillers:
                fillers.popleft()[0]()

            # ---------------- tail: out-proj for qh=3 ----------------
            ps_rotate["on"] = True
            if NQ == NSPLIT:
                for m in range(12, 16):
                    for (n0, nw) in ((0, 512), (512, 256)):
                        piece_op(m, n0, nw)()

            if debug:
                dKT = nc.dram_tensor("dKT", [128, NPR, S], BF, kind="ExternalOutput")
                dQT = nc.dram_tensor("dQT", [128, NH, SQ], BF, kind="ExternalOutput")
                dV8 = nc.dram_tensor("dV8", [128, NH, NJ, 128], F8, kind="ExternalOutput")
                dacc = nc.dram_tensor("dacc", [128, 12, 512], F32, kind="ExternalOutput")
                dy6 = nc.dram_tensor("dy6", [128, NH, SQ], BF, kind="ExternalOutput")
                nc.sync.dma_start(dKT[:], KT[:])
                nc.sync.dma_start(dQT[:], QTz[:])
                nc.sync.dma_start(dV8[:], V8[:])
                nc.sync.dma_start(dacc[:], acc[:])
                nc.sync.dma_start(dy6[:], y6[:])

    nc.finalize()
    return nc


_NC_CACHE = None


def make_in_maps(x, wq, bq, wk, bk, wv, ww):
    x = np.ascontiguousarray(np.asarray(x, dtype=np.float32))
    xT_full = np.ascontiguousarray(x[0].T).astype(ml_dtypes.bfloat16)  # [D, S]
    in_maps = []
    for core in range(8):
        g, c = core // NC, core % NC
        gs = slice(g * DH, (g + 1) * DH)
        wkp = np.zeros((D, DHP), np.float32)
        wkp[:, 0:DH] = wk[gs, :].T
        wqp = np.zeros((D, DHP), np.float32)
        wqp[:, 0:DH] = wq[gs, :].T
        bqp = np.zeros((256,), np.float32)
        bqp[0:DH] = bq[gs]
        bkp = np.zeros((256,), np.float32)
        bkp[0:DH] = bk[gs]
        in_maps.append({
            "xT": xT_full,
            "xqT": np.ascontiguousarray(xT_full[:, c * SQ:(c + 1) * SQ]),
            "wqT": wqp.astype(ml_dtypes.bfloat16),
            "wkT": wkp.astype(ml_dtypes.bfloat16),
            "wvT": np.ascontiguousarray(wv[gs, :].T * VSCALE).astype(ml_dtypes.bfloat16),
            "wwT": np.ascontiguousarray(ww[:, gs].T / VSCALE).astype(ml_dtypes.bfloat16),
            "bq": np.ascontiguousarray(bqp.reshape(NPR, 128).T).astype(np.float32),
            "bk": np.ascontiguousarray(bkp.reshape(NPR, 128).T).astype(np.float32),
        })
    return in_maps


def kernel(x, wq, bq, wk, bk, wv, bv, ww, bw):
    global _NC_CACHE
    if _NC_CACHE is None:
        _NC_CACHE = build_nc()
    nc = _NC_CACHE

    in_maps = make_in_maps(x, wq, bq, wk, bk, wv, ww)
    res = run_bass_kernel_spmd(nc, in_maps, core_ids=list(range(8)))

    const_row = (bv @ ww.T + bw).astype(np.float32)  # [768]
    out = np.empty((1, S, D), dtype=np.float32)
    for c in range(NC):
        acc_out = res.results[c]["out"].copy()
        for g in range(1, NG):
            acc_out += res.results[g * NC + c]["out"]
        out[0, c * SQ:(c + 1) * SQ, :] = acc_out + const_row
    return out


# revision 52
# speedup vs baseline: 1.0690x; 1.0690x over previous
"""Multi-head attention (B=1, S=4096, D=768, H=12, Hd=64) on 8 trn2 cores.

Sharding (v3): 4 head-groups (3 heads = 192 dims, Megatron column-split
wq/wk/wv, row-split ww) x 2 query-chunks (2048 rows).  core = g*2 + c.
Each core returns a partial output [2048, 768]; host sums the 4 group
partials per chunk and adds (bv @ ww.T + bw).  Fewer heads per group halves
the replicated K/V projection work versus a 2x6 split, which keeps the PE
under the ACT (exp) budget in every phase.

Pipeline design (ACT-bound fused schedule):
  - Projections / scores in bf16.  K/Q weights are zero-padded to 256 cols
    so the 3 heads pack into 2 K=128 pairs (head 2 rides rows 0-63 of pair
    1, upper half zero).
  - attnV in fp8e4 DoubleRow (2 key-tiles per instruction): V8 holds
    fp8(32*V) rows per key with a ones column at 64 and 63 pad cols (dual
    fp8 ldweights needs all 128 weight columns); pt = fp8(8*exp(s/8))
    written directly by the ACT exp.  y6 = 32*out; ww is pre-divided by 32
    on the host.
  - The exp stream on ACT is the binding engine.  The key axis runs in
    NSPLIT=4 quarters; each quarter has 12 rounds (qh 0-3, head 0-2) of 4
    cs=2 chunks through a 3-deep [128,2,512] psum rotation.  attnV pairs
    and the per-round spill are queued and emitted one chunk late so the
    in-order PE queue never blocks on a just-issued exp.  K/V projection
    blocks, the Q projection, and the out-projection run as paced filler
    pieces inside the quarters; `require` force-emits any producer a
    consumer needs (program order = dependency order).  attnV accumulates
    per-quarter in a shared 2-buf psum pool and spills into acc (f32).
  - psum: scores 3x2 banks + 2 shared o/filler banks = 8.
"""

import sys

if "/opt/trn_rl_repo" not in sys.path:
    sys.path.insert(0, "/opt/trn_rl_repo")

import math
import os
from collections import deque

import numpy as np
import ml_dtypes

import concourse.bacc as bacc
import concourse.mybir as mybir
import concourse.tile as tile
from concourse.bass_utils import run_bass_kernel_spmd
from concourse.vector_clock import ScopedClock

F32 = mybir.dt.float32
BF = mybir.dt.bfloat16
F8 = mybir.dt.float8e4
AF = mybir.ActivationFunctionType
DR = mybir.MatmulPerfMode.DoubleRow

S = 4096          # sequence length
D = 768           # model dim
NG = 4            # head groups (cores axis 1)
NC = 2            # query chunks (cores axis 2)
DH = D // NG      # dims per group = 192
DHP = 256         # padded dims (2 K=128 pairs)
NPR = 2           # K=128 pairs per group
NH = 3            # heads per group
SQ = S // NC      # queries per core = 2048
NQH = SQ // 512   # 512-query rounds per head = 4
KO = D // 128     # contraction subtiles = 6
NJ = S // 128     # key tiles = 32
SCALE = 0.125     # 1/sqrt(64)
LN8 = float(math.log(8.0))
VSCALE = 32.0     # folded into wv (1/VSCALE into ww); keeps |VSCALE*v| well
                  # below the TRN e4m3 max of 240 (DVE f32->fp8 conversion
                  # overflows instead of saturating)

PROBE = os.environ.get("PROBE", "")   # unused; kept for probe scripts

NSPLIT = 4        # key-axis quarters
JQ = NJ // NSPLIT           # j-tiles per quarter = 8
NPAIR = JQ // 2             # DoubleRow pairs per round-quarter = 4
QCHUNKS = [2, 2, 2, 2]      # exp chunk sizes covering JQ j-tiles
SC_BUFS = 3
SCW = max(QCHUNKS)
ROUNDS = [(qh, h) for qh in range(NQH) for h in range(NH)]  # 12 per quarter

_PATCHED = False


def _patch_drain():
    """walrus in this container rejects >1 sync-wait per instruction
    ("Too many sync wait commands").  TileContext's tail drain aggregates one
    wait per live tile semaphore; redistribute them one-per-nop.  (Bacc's
    generate_event_semaphores handles the rest of the kernel.)"""
    global _PATCHED
    if _PATCHED:
        return
    _PATCHED = True

    def _drain_and_barrier(self, tick_clock, wait_clock):
        nc = self.nc
        drain_inst = nc.sync.drain()
        wait_clock.add_sem_waits(
            drain_inst.ins, ScopedClock({None: tick_clock.global_clock})
        )
        si = drain_inst.ins.sync_info
        waits = list(si.on_wait) if si is not None else []
        if len(waits) > 1:
            drain_inst.ins.sync_info = mybir.SyncInfo(
                on_wait=[waits[0]], on_update=list(si.on_update)
            )
            for w in waits[1:]:
                nop = nc.sync.nop(nofuse=True)
                nop.ins.sync_info = mybir.SyncInfo(on_wait=[w], on_update=[])
        nc.all_engine_barrier()
        assert self.sems is not None
        popped = nc._tile_sem_poison_stack.pop()
        assert popped is self._sem_poison
        nc.clear_and_free_semaphores(list(self.sems.allocated().values()))
        nc.all_engine_barrier()

    tile.TileContext._drain_and_barrier = _drain_and_barrier


def build_nc(loop_n=None, debug=False):
    _patch_drain()
    nc = bacc.Bacc("TRN2", target_bir_lowering=False)

    xT = nc.dram_tensor("xT", [D, S], BF, kind="ExternalInput")
    xqT = nc.dram_tensor("xqT", [D, SQ], BF, kind="ExternalInput")
    wqT = nc.dram_tensor("wqT", [D, DHP], BF, kind="ExternalInput")  # padded
    wkT = nc.dram_tensor("wkT", [D, DHP], BF, kind="ExternalInput")  # padded
    wvT = nc.dram_tensor("wvT", [D, DH], BF, kind="ExternalInput")   # x VSCALE
    wwT = nc.dram_tensor("wwT", [DH, D], BF, kind="ExternalInput")   # / VSCALE
    bq = nc.dram_tensor("bq", [128, NPR], F32, kind="ExternalInput")
    bk = nc.dram_tensor("bk", [128, NPR], F32, kind="ExternalInput")
    out = nc.dram_tensor("out", [SQ, D], F32, kind="ExternalOutput")

    xT_r = xT.rearrange("(ko p) n -> p ko n", p=128)
    xqT_r = xqT.rearrange("(ko p) n -> p ko n", p=128)
    wqT_r = wqT.rearrange("(ko p) m -> p ko m", p=128)
    wkT_r = wkT.rearrange("(ko p) m -> p ko m", p=128)
    wvT_r = wvT.rearrange("(ko p) m -> p ko m", p=128)
    ww6_r = wwT.rearrange("(h l) o -> l h o", l=64)   # [64, 3, 768]

    with tile.TileContext(nc) as tc:
        import contextlib

        with contextlib.ExitStack() as ctx:
            if loop_n is not None:
                ctx.enter_context(tc.For_i(0, loop_n, 1))
            persist = ctx.enter_context(tc.tile_pool(name="persist", bufs=1))
            KT = persist.tile([128, NPR, S], BF)        # 16KB/part
            QTz = persist.tile([128, NH, SQ], BF)       # 12KB/part
            V8 = persist.tile([128, NH, NJ, 128], F8)   # 12KB/part
            acc = persist.tile([128, 12, 512], F32)     # 24KB/part
            ptr = persist.tile([128, 2, 6, 512], F8)    # exp rings, 6KB/part
            y6 = persist.tile([128, NH, SQ], BF)        # 12KB/part
            ww6 = persist.tile([128, NH, D], BF)        # 4.5KB/part
            lnb = persist.tile([128, 1], F32)
            bq_sb = persist.tile([128, NPR], F32)
            bk_sb = persist.tile([128, NPR], F32)

            w_pool = ctx.enter_context(tc.tile_pool(name="w", bufs=1))
            wk_sb = w_pool.tile([128, KO, DHP], BF)
            wv_sb = w_pool.tile([128, KO, DH], BF)
            wq_sb = w_pool.tile([128, KO, DHP], BF)

            xs = ctx.enter_context(tc.tile_pool(name="xs", bufs=3))
            ob_pool = ctx.enter_context(tc.tile_pool(name="ob", bufs=2))
            bc_pool = ctx.enter_context(tc.tile_pool(name="bc", bufs=2))

            sc_pool = ctx.enter_context(
                tc.tile_pool(name="sc", bufs=SC_BUFS, space="PSUM"))
            # shared 2-buf pool for attnV accumulators AND filler psum
            ok_pool = ctx.enter_context(
                tc.tile_pool(name="ok", bufs=2, space="PSUM"))

            # ---------------- init + weight DMA ----------------
            nc.sync.dma_start(wk_sb[:], wkT_r[:])
            nc.sync.dma_start(bk_sb[:], bk[:])
            nc.vector.memset(lnb[:], LN8)
            # zero inits on gpsimd (idle early); QTz zero halves kill the
            # cross-head term of the pair-packed K=128 scores matmuls
            nc.gpsimd.memset(QTz[64:128, 0, :], 0.0)
            nc.gpsimd.memset(QTz[0:64, 1, :], 0.0)
            nc.gpsimd.memset(QTz[64:128, 2, :], 0.0)
            # upper halves zero: out-proj runs K=128 on single heads
            nc.gpsimd.memset(y6[64:128, :, :], 0.0)
            nc.gpsimd.memset(ww6[64:128, :, :], 0.0)

            # ---------------- pieces ----------------
            ready = set()
            ps_rotate = {"on": False, "i": 0}

            def next_ps(name):
                if not ps_rotate["on"]:
                    return ok_pool.tile([128, 512], F32, tag="ok", name=name)
                i = ps_rotate["i"] = ps_rotate["i"] + 1
                if i % 2 == 0:
                    return ok_pool.tile([128, 512], F32, tag="ok", name=name)
                sc = sc_pool.tile([128, SCW, 512], F32, tag="sc", name=name)
                return sc[:, 0, :]

            xb_tiles = {}

            def piece_dma_block(n):
                def go():
                    xb = xs.tile([128, KO, 512], BF, tag="xb", name=f"xb{n}")
                    xb_tiles[n] = xb
                    nc.sync.dma_start(xb[:], xT_r[:, :, n * 512:(n + 1) * 512])
                    # fp8 pad cols + ones col for this block's j-tiles
                    nc.gpsimd.memset(V8[:, :, 4 * n:4 * n + 4, 64:128], 0.0)
                    nc.gpsimd.memset(V8[:, :, 4 * n:4 * n + 4, 64:65], 1.0)
                return go

            def piece_k(n, p):
                def go():
                    xb = xb_tiles[n]
                    ps = next_ps(f"psk{n}_{p}")
                    for ko in range(KO):
                        nc.tensor.matmul(
                            ps[:], wk_sb[:, ko, p * 128:(p + 1) * 128],
                            xb[:, ko, :],
                            start=(ko == 0), stop=(ko == KO - 1),
                        )
                    nc.vector.tensor_scalar_add(
                        KT[:, p, n * 512:(n + 1) * 512], ps[:],
                        bk_sb[:, p:p + 1],
                    )
                    ready.add(("K", n, p))
                return go

            def piece_v(n, j4):
                def go():
                    xb = xb_tiles[n]
                    ps = next_ps(f"psv{n}_{j4}")
                    for ko in range(KO):
                        nc.tensor.matmul(
                            ps[:, :DH],
                            xb[:, ko, j4 * 128:(j4 + 1) * 128],
                            wv_sb[:, ko, :],
                            start=(ko == 0), stop=(ko == KO - 1),
                        )
                    nc.vector.tensor_copy(
                        V8[:, :, 4 * n + j4, 0:64],
                        ps[:, 0:DH].rearrange("l (h c) -> l h c", c=64),
                    )
                    ready.add(("V", 4 * n + j4))
                return go

            def block_pieces(n):
                ps = [piece_dma_block(n)]
                for p in range(NPR):
                    ps.append(piece_k(n, p))
                for j4 in range(4):
                    ps.append(piece_v(n, j4))
                return ps

            xq_tiles = {}

            def piece_qproj_dma(nq):
                def go():
                    xqb = xs.tile([128, KO, 512], BF, tag="xb",
                                  name=f"xqb{nq}")
                    xq_tiles[nq] = xqb
                    nc.sync.dma_start(
                        xqb[:], xqT_r[:, :, nq * 512:(nq + 1) * 512])
                return go

            def piece_qproj_p(nq, p):
                def go():
                    xqb = xq_tiles[nq]
                    nqs = slice(nq * 512, (nq + 1) * 512)
                    psq = next_ps(f"psq{nq}_{p}")
                    for ko in range(KO):
                        nc.tensor.matmul(
                            psq[:], wq_sb[:, ko, p * 128:(p + 1) * 128],
                            xqb[:, ko, :],
                            start=(ko == 0), stop=(ko == KO - 1),
                        )
                    nc.vector.tensor_scalar_add(
                        QTz[0:64, 2 * p, nqs], psq[0:64, :],
                        bq_sb[0:64, p:p + 1],
                    )
                    if p == 0:
                        nc.vector.tensor_scalar_add(
                            QTz[64:128, 1, nqs], psq[64:128, :],
                            bq_sb[64:128, p:p + 1],
                        )
                    ready.add(("Q", nq, p))
                return go

            ob_tiles = {}

            def piece_op(m, n0, nw):
                # out-projection for m-tile cols [n0, n0+nw), all 3 heads
                def go():
                    ms = slice(m * 128, (m + 1) * 128)
                    ps = next_ps(f"op{m}_{n0}")
                    for h in range(NH):
                        nc.tensor.matmul(
                            ps[:, :nw],
                            y6[:, h, ms],
                            ww6[:, h, n0:n0 + nw],
                            start=(h == 0), stop=(h == NH - 1),
                        )
                    if n0 == 0:
                        ob_tiles[m] = ob_pool.tile(
                            [128, D], F32, tag="ob", name=f"ob{m}")
                    ob = ob_tiles[m]
                    nc.vector.tensor_copy(ob[:, n0:n0 + nw], ps[:, :nw])
                    if n0 + nw == D:
                        nc.sync.dma_start(out[ms, :], ob[:])
                return go

            COST_DMA = 200
            COST_K = KO * 512
            COST_V = KO * DH
            COST_QP = KO * 512

            # ---------------- lead-in ----------------
            ps_rotate["on"] = True
            pieces0 = block_pieces(0)
            pieces0[0]()                        # xb0 DMA first in queue
            nc.sync.dma_start(wv_sb[:], wvT_r[:])
            nc.sync.dma_start(wq_sb[:], wqT_r[:])
            nc.sync.dma_start(bq_sb[:], bq[:])
            for piece in pieces0[1:]:
                piece()
            piece_qproj_dma(0)()
            piece_qproj_p(0, 0)()
            pieces1 = block_pieces(1)
            pieces1[0]()                        # xb1 DMA
            for piece in pieces1[1:1 + NPR]:    # K of block 1 (scores j4-7)
                piece()

            # ---------------- fused attention quarters ----------------
            ps_rotate["on"] = False
            NQ = int(os.environ.get("NQ", NSPLIT))  # timing probe: truncate
            pend = deque()     # (chunk-emitted, closure) attnV/spill queue
            gchunk = [0]
            fillers = deque()  # (closure, cost, min_round_gate) — persistent

            def require(marker):
                # force-emit fillers until the producer of `marker` has been
                # emitted (program order = dependency order)
                while marker not in ready and fillers:
                    fillers.popleft()[0]()
                assert marker in ready, f"missing producer {marker}"

            for q in range(NQ):
                if q == 0:
                    # deferred lead pieces first: V of block 1 (attnV pairs
                    # 2-3 of round 0) and Q head 2 (round 2), then Q for qh1
                    # (round 3), block 2, and the later Q groups with block 3
                    for piece in pieces1[1 + NPR:]:
                        fillers.append((piece, COST_V, 0))
                    fillers.append((piece_qproj_p(0, 1), COST_QP, 0))
                    fillers.append((piece_qproj_dma(1), COST_DMA, 0))
                    fillers.append((piece_qproj_p(1, 0), COST_QP, 0))
                    fillers.append((piece_qproj_p(1, 1), COST_QP, 0))
                    fillers.append((piece_dma_block(2), COST_DMA, 0))
                    for p in range(NPR):
                        fillers.append((piece_k(2, p), COST_K, 0))
                    fillers.append((piece_qproj_dma(2), COST_DMA, 0))
                    fillers.append((piece_qproj_p(2, 0), COST_QP, 0))
                    fillers.append((piece_qproj_p(2, 1), COST_QP, 0))
                    for j4 in range(4):
                        fillers.append((piece_v(2, j4), COST_V, 0))
                    fillers.append((piece_dma_block(3), COST_DMA, 0))
                    for p in range(NPR):
                        fillers.append((piece_k(3, p), COST_K, 0))
                    fillers.append((piece_qproj_dma(3), COST_DMA, 0))
                    fillers.append((piece_qproj_p(3, 0), COST_QP, 0))
                    fillers.append((piece_qproj_p(3, 1), COST_QP, 0))
                    for j4 in range(4):
                        fillers.append((piece_v(3, j4), COST_V, 0))
                elif q < NSPLIT - 1:
                    if q == 1:
                        nc.sync.dma_start(ww6[0:64, :, :], ww6_r[:])
                    na, nb = 2 * q + 2, 2 * q + 3
                    # both DMAs and K pieces ahead of the V pieces: the next
                    # quarter's scores depend on K, and the second DMA
                    # overlaps the first block's compute
                    fillers.append((piece_dma_block(na), COST_DMA, 0))
                    for p in range(NPR):
                        fillers.append((piece_k(na, p), COST_K, 0))
                    fillers.append((piece_dma_block(nb), COST_DMA, 0))
                    for j4 in range(4):
                        fillers.append((piece_v(na, j4), COST_V, 0))
                    for p in range(NPR):
                        fillers.append((piece_k(nb, p), COST_K, 0))
                    for j4 in range(4):
                        fillers.append((piece_v(nb, j4), COST_V, 0))
                else:
                    # out-proj for qh 0-2 as fillers gated on their rounds;
                    # qh 3 runs in the tail
                    for m in range(12):
                        for (n0, nw) in ((0, 512), (512, 256)):
                            fillers.append(
                                (piece_op(m, n0, nw), NH * nw,
                                 3 * (m // 4) + 3))
                total_cost = sum(c for _, c, _ in fillers)
                n_slots = 12 * len(QCHUNKS)
                budget_rate = total_cost / n_slots
                budget = 0.0

                for r, (qh, h) in enumerate(ROUNDS):
                    kp = h >> 1
                    qs = slice(qh * 512, (qh + 1) * 512)
                    ring = ptr[:, r % 2, :, :]
                    require(("Q", qh, kp))
                    ohold = {}

                    def mk_pair(t_l, q=q, r=r, h=h, ring=ring, ohold=ohold):
                        def go():
                            jg = q * JQ + 2 * t_l
                            require(("V", jg))
                            require(("V", jg + 1))
                            if t_l == 0:
                                ohold["t"] = ok_pool.tile(
                                    [128, 512], F32, tag="ok",
                                    name=f"o{q}_{r}")
                            nc.tensor.matmul(
                                ohold["t"][:],
                                V8[:, h, jg:jg + 2, :],
                                ring[:, (2 * t_l) % 6:(2 * t_l) % 6 + 2, :],
                                start=(t_l == 0), stop=(t_l == NPAIR - 1),
                                perf_mode=DR,
                            )
                        return go

                    def mk_spill(q=q, r=r, h=h, qs=qs, ohold=ohold):
                        def go():
                            o_ps = ohold["t"]
                            if q == 0:
                                nc.vector.tensor_copy(
                                    acc[0:65, r, :], o_ps[0:65, :])
                            else:
                                nc.vector.tensor_add(
                                    acc[0:65, r, :], o_ps[0:65, :],
                                    acc[0:65, r, :])
                            if q == NSPLIT - 1:
                                dn = bc_pool.tile([1, 512], F32, tag="dn",
                                                  name=f"dn{r}")
                                nc.vector.tensor_copy(dn[:], acc[64:65, r, :])
                                bc = bc_pool.tile([64, 512], F32, tag="bc",
                                                  name=f"bc{r}")
                                nc.gpsimd.partition_broadcast(
                                    bc[:], dn[:], channels=64)
                                nc.vector.reciprocal(bc[:], bc[:])
                                nc.vector.tensor_mul(
                                    y6[0:64, h, qs], acc[0:64, r, :], bc[:])
                        return go

                    jc = 0
                    pair_emitted = 0
                    for c, cs in enumerate(QCHUNKS):
                        for t in range(cs):
                            require(("K", (q * JQ + jc + t) // 4, kp))
                        sc = sc_pool.tile([128, SCW, 512], F32, tag="sc")
                        for t in range(cs):
                            j = q * JQ + jc + t
                            nc.tensor.matmul(
                                sc[:, t, :],
                                KT[:, kp, j * 128:(j + 1) * 128],
                                QTz[:, h, qs],
                                start=True, stop=True,
                            )
                        slot = jc % 6
                        nc.scalar.activation(
                            ring[:, slot:slot + cs, :], sc[:, :cs, :],
                            AF.Exp, scale=SCALE, bias=lnb[:],
                        )
                        jc += cs
                        # queue attnV pairs completed by this chunk's exp;
                        # they pop a chunk later (possibly in the next round
                        # or quarter) so the in-order PE queue never blocks
                        # on an exp that was just issued
                        while 2 * (pair_emitted + 1) <= jc:
                            pend.append((gchunk[0], mk_pair(pair_emitted)))
                            pair_emitted += 1
                        if pair_emitted == NPAIR:
                            pend.append((gchunk[0], mk_spill()))
                            pair_emitted += 1
                        gchunk[0] += 1
                        while pend and pend[0][0] < gchunk[0] - 1:
                            pend.popleft()[1]()
                        if not fillers or fillers[0][2] <= r:
                            budget += budget_rate
                        while fillers and budget >= fillers[0][1] \
                                and fillers[0][2] <= r:
                            piece, cost, _ = fillers.popleft()
                            piece()
                            budget -= cost

            while pend:
                pend.popleft()[1]()
            while fillers:
                fillers.popleft()[0]()

            # ---------------- tail: out-proj for qh=3 ----------------
            ps_rotate["on"] = True
            if NQ == NSPLIT:
                for m in range(12, 16):
                    for (n0, nw) in ((0, 512), (512, 256)):
                        piece_op(m, n0, nw)()

            if debug:
                dKT = nc.dram_tensor("dKT", [128, NPR, S], BF, kind="ExternalOutput")
                dQT = nc.dram_tensor("dQT", [128, NH, SQ], BF, kind="ExternalOutput")
                dV8 = nc.dram_tensor("dV8", [128, NH, NJ, 128], F8, kind="ExternalOutput")
                dacc = nc.dram_tensor("dacc", [128, 12, 512], F32, kind="ExternalOutput")
                dy6 = nc.dram_tensor("dy6", [128, NH, SQ], BF, kind="ExternalOutput")
                nc.sync.dma_start(dKT[:], KT[:])
                nc.sync.dma_start(dQT[:], QTz[:])
                nc.sync.dma_start(dV8[:], V8[:])
                nc.sync.dma_start(dacc[:], acc[:])
                nc.sync.dma_start(dy6[:], y6[:])

    nc.finalize()
    return nc


_NC_CACHE = None


def make_in_maps(x, wq, bq, wk, bk, wv, ww):
    x = np.ascontiguousarray(np.asarray(x, dtype=np.float32))
    xT_full = np.ascontiguousarray(x[0].T).astype(ml_dtypes.bfloat16)  # [D, S]
    in_maps = []
    for core in range(8):
        g, c = core // NC, core % NC
        gs = slice(g * DH, (g + 1) * DH)
        wkp = np.zeros((D, DHP), np.float32)
        wkp[:, 0:DH] = wk[gs, :].T
        wqp = np.zeros((D, DHP), np.float32)
        wqp[:, 0:DH] = wq[gs, :].T
        bqp = np.zeros((256,), np.float32)
        bqp[0:DH] = bq[gs]
        bkp = np.zeros((256,), np.float32)
        bkp[0:DH] = bk[gs]
        in_maps.append({
            "xT": xT_full,
            "xqT": np.ascontiguousarray(xT_full[:, c * SQ:(c + 1) * SQ]),
            "wqT": wqp.astype(ml_dtypes.bfloat16),
            "wkT": wkp.astype(ml_dtypes.bfloat16),
            "wvT": np.ascontiguousarray(wv[gs, :].T * VSCALE).astype(ml_dtypes.bfloat16),
            "wwT": np.ascontiguousarray(ww[:, gs].T / VSCALE).astype(ml_dtypes.bfloat16),
            "bq": np.ascontiguousarray(bqp.reshape(NPR, 128).T).astype(np.float32),
            "bk": np.ascontiguousarray(bkp.reshape(NPR, 128).T).astype(np.float32),
        })
    return in_maps


def kernel(x, wq, bq, wk, bk, wv, bv, ww, bw):
    global _NC_CACHE
    if _NC_CACHE is None:
        _NC_CACHE = build_nc()
    nc = _NC_CACHE

    in_maps = make_in_maps(x, wq, bq, wk, bk, wv, ww)
    res = run_bass_kernel_spmd(nc, in_maps, core_ids=list(range(8)))

    const_row = (bv @ ww.T + bw).astype(np.float32)  # [768]
    out = np.empty((1, S, D), dtype=np.float32)
    for c in range(NC):
        acc_out = res.results[c]["out"].copy()
        for g in range(1, NG):
            acc_out += res.results[g * NC + c]["out"]
        out[0, c * SQ:(c + 1) * SQ, :] = acc_out + const_row
    return out
